# revision 3
# baseline (speedup 1.0000x reference)
"""Direct N-body gravitational acceleration on 8 Trainium2 NeuronCores.

Sharding: target-particle axis j split across the 8 cores (1024 targets
per core); every core holds the full (N,3) source positions.

Math (reference):
    z[i,j]   = |y_i - y_j|^2 + eps
    icd[i,j] = z^{-3/2}
    acc[j]   = G*m_j * (sum_i icd[i,j]*y_i  -  y_j * sum_i icd[i,j])

Per-core device pipeline (everything O(N^2) stays on-chip):
    mm1 (PE, K=5):  z[i,j] = a_i . b_j with a_i=(y_i, d2_i, 1),
                    b_j=(-2*y_j, 1, d2_j+eps)   -> PSUM [128, 1024]
    ACT:            Ln then Exp(scale=-1.5)     -> icd in SBUF
    mm2 (PE, K=128 per i-tile, PSUM-accumulated over all 64 i-tiles):
                    S[c,j] += sum_i yp[i,c]*icd[i,j],  yp = [y | 1]
Host does the O(N) prep (feature vectors) and the O(N) affine combine
    acc[j] = G*m_j*(S[0:3,j] - y_j*S[3,j]).
"""

import numpy as np

N = 8192
NCORES = 8
JL = N // NCORES  # 1024 local targets per core
P = 128
ITILES = N // P  # 64
EPS = np.float32(0.01 * 0.01)

_cache: dict = {}
LAST_RUN = None  # BassKernelResults of the most recent launch (for test.py)


def _build():
    import concourse.bacc as bacc
    import concourse.mybir as mybir
    import concourse.tile as tile

    f32 = mybir.dt.float32
    AF = mybir.ActivationFunctionType

    nc = bacc.Bacc("TRN2", target_bir_lowering=False, debug=False)
    aT = nc.dram_tensor("aT", [5, N], f32, kind="ExternalInput")
    bT = nc.dram_tensor("bT", [5, JL], f32, kind="ExternalInput")
    yp = nc.dram_tensor("yp", [N, 4], f32, kind="ExternalInput")
    S = nc.dram_tensor("S", [4, JL], f32, kind="ExternalOutput")

    with tile.TileContext(nc) as tc:
        with (
            tc.tile_pool(name="const", bufs=1) as cpool,
            tc.tile_pool(name="lnp", bufs=3) as lnpool,
            tc.tile_pool(name="icdp", bufs=3) as icdpool,
            tc.tile_pool(name="ps1", bufs=2, space="PSUM") as ps1pool,
            tc.tile_pool(name="ps2", bufs=1, space="PSUM") as ps2pool,
            tc.tile_pool(name="outp", bufs=1) as opool,
        ):
            aT_sb = cpool.tile([5, N], f32)
            nc.sync.dma_start(aT_sb[:], aT[:])
            bT_sb = cpool.tile([5, JL], f32)
            nc.sync.dma_start(bT_sb[:], bT[:])
            yp_sb = cpool.tile([P, ITILES, 4], f32)
            nc.sync.dma_start(yp_sb[:], yp.rearrange("(t p) c -> p t c", p=P))

            ps2a = ps2pool.tile([4, 512], f32)
            ps2b = ps2pool.tile([4, 512], f32)
            for it in range(ITILES):
                ps1 = ps1pool.tile([P, JL], f32)
                nc.tensor.matmul(
                    ps1[:, 0:512],
                    aT_sb[:, it * P : (it + 1) * P],
                    bT_sb[:, 0:512],
                    start=True,
                    stop=True,
                )
                nc.tensor.matmul(
                    ps1[:, 512:1024],
                    aT_sb[:, it * P : (it + 1) * P],
                    bT_sb[:, 512:1024],
                    start=True,
                    stop=True,
                )
                lnz = lnpool.tile([P, JL], f32)
                nc.scalar.activation(lnz[:], ps1[:], AF.Ln)
                icd = icdpool.tile([P, JL], f32)
                nc.scalar.activation(icd[:], lnz[:], AF.Exp, scale=-1.5)
                first, last = it == 0, it == ITILES - 1
                nc.tensor.matmul(
                    ps2a[:], yp_sb[:, it, :], icd[:, 0:512], start=first, stop=last
                )
                nc.tensor.matmul(
                    ps2b[:], yp_sb[:, it, :], icd[:, 512:1024], start=first, stop=last
                )
            S_sb = opool.tile([4, JL], f32)
            nc.vector.tensor_copy(S_sb[:, 0:512], ps2a[:])
            nc.vector.tensor_copy(S_sb[:, 512:1024], ps2b[:])
            nc.sync.dma_start(S[:], S_sb[:])
    nc.compile()
    return nc


def kernel(t, y, masses, G):
    global LAST_RUN
    from concourse.bass_utils import run_bass_kernel_spmd

    y = np.asarray(y, np.float32).reshape(N, 3)
    m = np.asarray(masses, np.float32).reshape(N)
    g = np.float32(np.asarray(G).reshape(()))

    d2 = (y * y).sum(1, dtype=np.float32)
    ones = np.ones(N, np.float32)
    aT = np.ascontiguousarray(np.stack([y[:, 0], y[:, 1], y[:, 2], d2, ones]))
    bT_full = np.ascontiguousarray(
        np.stack([-2 * y[:, 0], -2 * y[:, 1], -2 * y[:, 2], ones, d2 + EPS])
    )
    yp = np.ascontiguousarray(np.concatenate([y, ones[:, None]], axis=1))

    if "nc" not in _cache:
        _cache["nc"] = _build()
    nc = _cache["nc"]

    in_maps = [
        {
            "aT": aT,
            "bT": np.ascontiguousarray(bT_full[:, c * JL : (c + 1) * JL]),
            "yp": yp,
        }
        for c in range(NCORES)
    ]
    LAST_RUN = run_bass_kernel_spmd(nc, in_maps, core_ids=list(range(NCORES)))
    S = np.concatenate([r["S"] for r in LAST_RUN.results], axis=1)  # [4, N]
    acc = (g * m)[:, None] * (S[0:3].T - y * S[3][:, None])
    return acc.astype(np.float32)


# revision 9
# speedup vs baseline: 2.4886x; 2.4886x over previous
"""Direct N-body gravitational acceleration on 8 Trainium2 NeuronCores.

Sharding: target-particle axis j split across the 8 cores (1024 targets
per core); every core holds the full (N,3) source positions.

Math (reference):
    z[i,j]   = |y_i - y_j|^2 + eps
    icd[i,j] = z^{-3/2}
    acc[j]   = G*m_j * (sum_i icd[i,j]*y_i  -  y_j * sum_i icd[i,j])

Per-core device pipeline (everything O(N^2) stays on-chip):
    mm1 (PE): z[i,j] = a_i . b_j with a_i=(y_i, d2_i, 1),
        b_j=(-2*y_j, 1, d2_j+eps).  To avoid the 4x-cost fp32 matmul
        path, each fp32 feature is split into fp16 hi+mid halves
        (22-bit mantissa total) and the product expanded into the four
        cross terms -> one K=20 fp16 matmul, fp32-grade accuracy.
    ACT: Ln then Exp(scale=-1.5, bias=-ln 16) -> icd/16 written as fp16
        (icd spans [2e-3, 1e6]; /16 centers it inside fp16 range).
    mm2 (PE, fp16, K=128 per i-tile, PSUM-accumulated over 64 i-tiles):
        S[c,j] += sum_i yp[i,c]*icd[i,j], with yp = [y | 1] split into
        fp16 hi+lo (two accumulating matmuls) so the y_i - y_j
        cancellation survives quantization.
Host does the O(N) prep (feature splits) and the O(N) affine combine
    acc[j] = 16*G*m_j*(S[0:3,j] - y_j*S[3,j]).
"""

import numpy as np

N = 8192
NCORES = 8
JL = N // NCORES  # 1024 local targets per core
P = 128
ITILES = N // P  # 64
EPS = np.float32(0.01 * 0.01)
ICD_SCALE = 16.0  # icd stored as icd/16 in fp16
KF = 30  # feature rows after 3-way fp16 split (6 kept cross-product pairs)
LN_EPS = float(__import__('numpy').log(__import__('numpy').float32(0.01 * 0.01)))

_cache: dict = {}
LAST_RUN = None  # BassKernelResults of the most recent launch (for test.py)


def _build():
    import concourse.bacc as bacc
    import concourse.mybir as mybir
    import concourse.tile as tile

    f32 = mybir.dt.float32
    f16 = mybir.dt.float16
    AF = mybir.ActivationFunctionType

    nc = bacc.Bacc("TRN2", target_bir_lowering=False, debug=False)
    aT = nc.dram_tensor("aT", [KF, N], f16, kind="ExternalInput")
    bT = nc.dram_tensor("bT", [KF, JL], f16, kind="ExternalInput")
    yph = nc.dram_tensor("yph", [N, 4], f16, kind="ExternalInput")
    ypl = nc.dram_tensor("ypl", [N, 4], f16, kind="ExternalInput")
    S = nc.dram_tensor("S", [4, JL], f32, kind="ExternalOutput")

    with tile.TileContext(nc) as tc:
        with (
            tc.tile_pool(name="const", bufs=1) as cpool,
            tc.tile_pool(name="lnp", bufs=3) as lnpool,
            tc.tile_pool(name="icdp", bufs=3) as icdpool,
            tc.tile_pool(name="ps1", bufs=2, space="PSUM") as ps1pool,
            tc.tile_pool(name="ps2", bufs=1, space="PSUM") as ps2pool,
            tc.tile_pool(name="outp", bufs=1) as opool,
        ):
            aT_sb = cpool.tile([KF, N], f16)
            nc.sync.dma_start(aT_sb[:], aT[:])
            bT_sb = cpool.tile([KF, JL], f16)
            nc.sync.dma_start(bT_sb[:], bT[:])
            yph_sb = cpool.tile([P, ITILES, 4], f16)
            nc.sync.dma_start(yph_sb[:], yph.rearrange("(t p) c -> p t c", p=P))
            ypl_sb = cpool.tile([P, ITILES, 4], f16)
            nc.sync.dma_start(ypl_sb[:], ypl.rearrange("(t p) c -> p t c", p=P))
            exp_bias = cpool.tile([P, 1], f32)
            nc.gpsimd.memset(exp_bias[:], -float(np.log(ICD_SCALE)))

            ps2a = ps2pool.tile([4, 512], f32)
            ps2b = ps2pool.tile([4, 512], f32)
            for it in range(ITILES):
                ps1 = ps1pool.tile([P, JL], f32)
                lhs1 = aT_sb[:, it * P : (it + 1) * P]
                nc.tensor.matmul(
                    ps1[:, 0:512], lhs1, bT_sb[:, 0:512], start=True, stop=True
                )
                nc.tensor.matmul(
                    ps1[:, 512:1024], lhs1, bT_sb[:, 512:1024], start=True, stop=True
                )
                lnz = lnpool.tile([P, JL], f32)
                nc.scalar.activation(lnz[:], ps1[:], AF.Ln)
                nc.vector.tensor_scalar_max(lnz[:], lnz[:], LN_EPS)
                icd = icdpool.tile([P, JL], f16)
                nc.scalar.activation(
                    icd[:], lnz[:], AF.Exp, scale=-1.5, bias=exp_bias[:]
                )
                first, last = it == 0, it == ITILES - 1
                nc.tensor.matmul(
                    ps2a[:], yph_sb[:, it, :], icd[:, 0:512], start=first, stop=False
                )
                nc.tensor.matmul(
                    ps2a[:], ypl_sb[:, it, :], icd[:, 0:512], start=False, stop=last
                )
                nc.tensor.matmul(
                    ps2b[:], yph_sb[:, it, :], icd[:, 512:1024], start=first, stop=False
                )
                nc.tensor.matmul(
                    ps2b[:], ypl_sb[:, it, :], icd[:, 512:1024], start=False, stop=last
                )
            S_sb = opool.tile([4, JL], f32)
            nc.vector.tensor_copy(S_sb[:, 0:512], ps2a[:])
            nc.vector.tensor_copy(S_sb[:, 512:1024], ps2b[:])
            nc.sync.dma_start(S[:], S_sb[:])

    # Bacc's act-table pass resolves Ln via the "natural_log" set and Exp
    # via "exp_and_others", forcing a ~1.3us table swap per ACTIVATE.  Strip
    # the two functions from every set except the one that holds both so a
    # single hoisted load covers the whole kernel.
    import concourse.hw_specs as hw_specs

    orig_gat = hw_specs.get_activation_tables

    def gat_combined(arch):
        tables = orig_gat(arch)
        out = {}
        for name, funcs in tables.items():
            if name != "natural_log_exp_and_others":
                funcs = funcs - {AF.Ln, AF.Exp}
            out[name] = funcs
        return out

    bacc.get_activation_tables = gat_combined
    try:
        nc.compile()
    finally:
        bacc.get_activation_tables = orig_gat
    return nc


def _split16(x):
    hi = x.astype(np.float16)
    lo = (x - hi.astype(np.float32)).astype(np.float16)
    return hi, lo


def _split16_3(x):
    h = x.astype(np.float16)
    r = x - h.astype(np.float32)
    m = r.astype(np.float16)
    l = (r - m.astype(np.float32)).astype(np.float16)
    return h, m, l


def kernel(t, y, masses, G):
    global LAST_RUN
    from concourse.bass_utils import run_bass_kernel_spmd

    y = np.asarray(y, np.float32).reshape(N, 3)
    m = np.asarray(masses, np.float32).reshape(N)
    g = np.float32(np.asarray(G).reshape(()))

    d2 = (y * y).sum(1, dtype=np.float32)
    ones = np.ones(N, np.float32)
    a = np.stack([y[:, 0], y[:, 1], y[:, 2], d2, ones])  # [5, N] fp32
    b = np.stack([-2 * y[:, 0], -2 * y[:, 1], -2 * y[:, 2], ones, d2 + EPS])
    ah, am, al = _split16_3(a)
    bh, bm, bl = _split16_3(b)
    # (ah+am+al).(bh+bm+bl) expanded, keeping pairs whose product can reach
    # ~2^-22 of z: (h,h) (h,m) (m,h) (h,l) (l,h) (m,m); dropped terms < 2^-33.
    aT20 = np.ascontiguousarray(np.concatenate([ah, ah, am, ah, al, am], axis=0))
    bT20_full = np.concatenate([bh, bm, bh, bl, bh, bm], axis=0)  # [30, N]
    yp = np.concatenate([y, ones[:, None]], axis=1)  # [N, 4] fp32
    yph, ypl = _split16(yp)
    yph = np.ascontiguousarray(yph)
    ypl = np.ascontiguousarray(ypl)

    if "nc" not in _cache:
        _cache["nc"] = _build()
    nc = _cache["nc"]

    in_maps = [
        {
            "aT": aT20,
            "bT": np.ascontiguousarray(bT20_full[:, c * JL : (c + 1) * JL]),
            "yph": yph,
            "ypl": ypl,
        }
        for c in range(NCORES)
    ]
    LAST_RUN = run_bass_kernel_spmd(nc, in_maps, core_ids=list(range(NCORES)))
    S = np.concatenate([r["S"] for r in LAST_RUN.results], axis=1)  # [4, N]
    acc = (np.float32(ICD_SCALE) * g * m)[:, None] * (S[0:3].T - y * S[3][:, None])
    return acc.astype(np.float32)


# revision 10
# speedup vs baseline: 2.7419x; 1.1018x over previous
"""Direct N-body gravitational acceleration on 8 Trainium2 NeuronCores.

Sharding: target-particle axis j split across the 8 cores (1024 targets
per core); every core holds the full (N,3) source positions.

Math (reference):
    z[i,j]   = |y_i - y_j|^2 + eps
    icd[i,j] = z^{-3/2}
    acc[j]   = G*m_j * (sum_i icd[i,j]*y_i  -  y_j * sum_i icd[i,j])

Per-core device pipeline (everything O(N^2) stays on-chip):
    mm1 (PE): z[i,j] = a_i . b_j with a_i=(y_i, d2_i, 1),
        b_j=(-2*y_j, 1, d2_j+eps).  To avoid the 4x-cost fp32 matmul
        path each fp32 feature is 3-way fp16 split and the product
        expanded into 6 cross terms -> one K=30 fp16 matmul with
        fp32-grade accuracy.
    ACT: t = Abs_reciprocal_sqrt(z) = z^{-1/2}  (HW-measured 4.4e-5
        max rel err), one pass.
    DVE (custom fused op): icd/16 = min(t, 100)^3 / 16 written as fp16.
        The clamp bounds fp16 at 62500 (z rounding can dip below eps on
        the diagonal; true off-diagonal pairs sit well above the clamp).
    mm2 (PE, fp16, K=128 per i-tile, PSUM-accumulated over 64 i-tiles):
        S[c,j] += sum_i yp[i,c]*icd[i,j], with yp = [y | 1] split into
        fp16 hi+lo halves packed as one [128, 8] weight (the y_i - y_j
        cancellation must survive quantization) -> one matmul per
        (i-tile, j-half).
Host does the O(N) prep (feature splits) and the O(N) affine combine
    acc[j] = 16*G*m_j*(S[0:3,j] - y_j*S[3,j]) with S = S_hi + S_lo.
"""

import numpy as np

N = 8192
NCORES = 8
JL = N // NCORES  # 1024 local targets per core
P = 128
ITILES = N // P  # 64
EPS = np.float32(0.01 * 0.01)
ICD_SCALE = 16.0  # icd stored as icd/16 in fp16
T_CLAMP = 100.0  # = EPS**-0.5; min(t, clamp)^3/16 = 62500 < fp16 max
KF = 30  # feature rows after 3-way fp16 split (6 kept cross-product pairs)

_cache: dict = {}
LAST_RUN = None  # BassKernelResults of the most recent launch (for test.py)


def _register_cube_op():
    """Register a fused clamp+cube+scale custom DVE op (the documented
    dve_ops extension point, applied at runtime since the repo is read-only):
    out = min(in0, s0)^3 * s1."""
    import concourse.dve_ops as dve_ops
    from concourse.dve_spec import Spec, Src0, C0, C1, lower, minn, sq
    from concourse.dve_uop import DveOpSpec

    name = "CUBE_CLAMP_SCALE_NB"
    for op in dve_ops.OPS:
        if op.name == name:
            return op

    m = minn(Src0, C0)
    spec = Spec(
        body=sq(m) * m * C1,
        reference=lambda in0, in1, s0, s1, imm2: (
            np.minimum(in0.astype(np.float32), s0) ** 3 * s1
        ),
    )
    row = dve_ops._CUSTOM_DVE_ROW_BASE + len(dve_ops.OPS)
    shas = {}
    for ver in ("v3", "v4"):
        try:
            uops = lower(spec, ver=ver)
        except Exception:
            continue
        shas[ver] = DveOpSpec(name=name, opcode=row, uops=uops, rd1_en=False).sha(ver)
    op = dve_ops.DveOp(name, spec, subdim=False, uops_sha=shas)
    dve_ops.OPS.append(op)
    dve_ops.CUSTOM_DVE_SPECS[name] = spec
    dve_ops._SUB_OPCODE_FOR_NAME[name] = row
    return op


def _build():
    import concourse.bacc as bacc
    import concourse.mybir as mybir
    import concourse.tile as tile

    f32 = mybir.dt.float32
    f16 = mybir.dt.float16
    AF = mybir.ActivationFunctionType

    cube_op = _register_cube_op()

    nc = bacc.Bacc("TRN2", target_bir_lowering=False, debug=False)
    aT = nc.dram_tensor("aT", [KF, N], f16, kind="ExternalInput")
    bT = nc.dram_tensor("bT", [KF, JL], f16, kind="ExternalInput")
    ypc = nc.dram_tensor("ypc", [N, 8], f16, kind="ExternalInput")
    S = nc.dram_tensor("S", [8, JL], f32, kind="ExternalOutput")

    with tile.TileContext(nc) as tc:
        with (
            tc.tile_pool(name="const", bufs=1) as cpool,
            tc.tile_pool(name="tp", bufs=3) as tpool,
            tc.tile_pool(name="icdp", bufs=3) as icdpool,
            tc.tile_pool(name="ps1", bufs=2, space="PSUM") as ps1pool,
            tc.tile_pool(name="ps2", bufs=1, space="PSUM") as ps2pool,
            tc.tile_pool(name="outp", bufs=1) as opool,
        ):
            aT_sb = cpool.tile([KF, N], f16)
            nc.sync.dma_start(aT_sb[:], aT[:])
            bT_sb = cpool.tile([KF, JL], f16)
            nc.sync.dma_start(bT_sb[:], bT[:])
            ypc_sb = cpool.tile([P, ITILES, 8], f16)
            nc.sync.dma_start(ypc_sb[:], ypc.rearrange("(t p) c -> p t c", p=P))

            ps2a = ps2pool.tile([8, 512], f32)
            ps2b = ps2pool.tile([8, 512], f32)
            for it in range(ITILES):
                ps1 = ps1pool.tile([P, JL], f32)
                lhs1 = aT_sb[:, it * P : (it + 1) * P]
                nc.tensor.matmul(
                    ps1[:, 0:512], lhs1, bT_sb[:, 0:512], start=True, stop=True
                )
                nc.tensor.matmul(
                    ps1[:, 512:1024], lhs1, bT_sb[:, 512:1024], start=True, stop=True
                )
                t = tpool.tile([P, JL], f32)
                nc.scalar.activation(t[:], ps1[:], AF.Abs_reciprocal_sqrt)
                icd = icdpool.tile([P, JL], f16)
                nc.vector._custom_dve(
                    cube_op, out=icd[:], in0=t[:], s0=T_CLAMP, s1=1.0 / ICD_SCALE
                )
                first, last = it == 0, it == ITILES - 1
                nc.tensor.matmul(
                    ps2a[:], ypc_sb[:, it, :], icd[:, 0:512], start=first, stop=last
                )
                nc.tensor.matmul(
                    ps2b[:], ypc_sb[:, it, :], icd[:, 512:1024], start=first, stop=last
                )
            S_sb = opool.tile([8, JL], f32)
            nc.vector.tensor_copy(S_sb[:, 0:512], ps2a[:])
            nc.vector.tensor_copy(S_sb[:, 512:1024], ps2b[:])
            nc.sync.dma_start(S[:], S_sb[:])
    nc.compile()
    return nc


def _split16(x):
    hi = x.astype(np.float16)
    lo = (x - hi.astype(np.float32)).astype(np.float16)
    return hi, lo


def _split16_3(x):
    h = x.astype(np.float16)
    r = x - h.astype(np.float32)
    m = r.astype(np.float16)
    l = (r - m.astype(np.float32)).astype(np.float16)
    return h, m, l


def kernel(t, y, masses, G):
    global LAST_RUN
    from concourse.bass_utils import run_bass_kernel_spmd

    y = np.asarray(y, np.float32).reshape(N, 3)
    m = np.asarray(masses, np.float32).reshape(N)
    g = np.float32(np.asarray(G).reshape(()))

    d2 = (y * y).sum(1, dtype=np.float32)
    ones = np.ones(N, np.float32)
    a = np.stack([y[:, 0], y[:, 1], y[:, 2], d2, ones])  # [5, N] fp32
    b = np.stack([-2 * y[:, 0], -2 * y[:, 1], -2 * y[:, 2], ones, d2 + EPS])
    ah, am, al = _split16_3(a)
    bh, bm, bl = _split16_3(b)
    # (ah+am+al).(bh+bm+bl) expanded, keeping pairs whose product can reach
    # ~2^-22 of z: (h,h) (h,m) (m,h) (h,l) (l,h) (m,m); dropped terms < 2^-33.
    aT30 = np.ascontiguousarray(np.concatenate([ah, ah, am, ah, al, am], axis=0))
    bT30_full = np.concatenate([bh, bm, bh, bl, bh, bm], axis=0)  # [30, N]
    yp = np.concatenate([y, ones[:, None]], axis=1)  # [N, 4] fp32
    yph, ypl = _split16(yp)
    ypc = np.ascontiguousarray(np.concatenate([yph, ypl], axis=1))  # [N, 8]

    if "nc" not in _cache:
        _cache["nc"] = _build()
    nc = _cache["nc"]

    in_maps = [
        {
            "aT": aT30,
            "bT": np.ascontiguousarray(bT30_full[:, c * JL : (c + 1) * JL]),
            "ypc": ypc,
        }
        for c in range(NCORES)
    ]
    LAST_RUN = run_bass_kernel_spmd(nc, in_maps, core_ids=list(range(NCORES)))
    S8 = np.concatenate([r["S"] for r in LAST_RUN.results], axis=1)  # [8, N]
    S = S8[0:4] + S8[4:8]
    acc = (np.float32(ICD_SCALE) * g * m)[:, None] * (S[0:3].T - y * S[3][:, None])
    return acc.astype(np.float32)


# revision 12
# speedup vs baseline: 2.7757x; 1.0123x over previous
"""Direct N-body gravitational acceleration on 8 Trainium2 NeuronCores.

Sharding: target-particle axis j split across the 8 cores (1024 targets
per core); every core holds the full (N,3) source positions.

Math (reference):
    z[i,j]   = |y_i - y_j|^2 + eps
    icd[i,j] = z^{-3/2}
    acc[j]   = G*m_j * (sum_i icd[i,j]*y_i  -  y_j * sum_i icd[i,j])

Per-core device pipeline (everything O(N^2) stays on-chip):
    mm1 (PE): z[i,j] = a_i . b_j with a_i=(y_i, d2_i, 1),
        b_j=(-2*y_j, 1, d2_j+eps).  To avoid the 4x-cost fp32 matmul
        path each fp32 feature is 3-way fp16 split and the product
        expanded into 6 cross terms -> one K=30 fp16 matmul with
        fp32-grade accuracy.
    ACT: t = Abs_reciprocal_sqrt(z) = z^{-1/2}  (HW-measured 4.4e-5
        max rel err), one pass.
    DVE (custom fused op): icd/16 = min(t, 100)^3 / 16 written as fp16.
        The clamp bounds fp16 at 62500 (z rounding can dip below eps on
        the diagonal; true off-diagonal pairs sit well above the clamp).
    mm2 (PE, fp16, K=128 per i-tile, PSUM-accumulated over 64 i-tiles):
        S[c,j] += sum_i yp[i,c]*icd[i,j], with yp = [y | 1] split into
        fp16 hi+lo halves packed as one [128, 8] weight (the y_i - y_j
        cancellation must survive quantization) -> one matmul per
        (i-tile, j-half).
Host does the O(N) prep (feature splits) and the O(N) affine combine
    acc[j] = 16*G*m_j*(S[0:3,j] - y_j*S[3,j]) with S = S_hi + S_lo.
"""

import numpy as np

N = 8192
NCORES = 8
JL = N // NCORES  # 1024 local targets per core
P = 128
ITILES = N // P  # 64
EPS = np.float32(0.01 * 0.01)
ICD_SCALE = 16.0  # icd stored as icd/16 in fp16
T_CLAMP = 100.0  # = EPS**-0.5; min(t, clamp)^3/16 = 62500 < fp16 max
KF = 30  # feature rows after 3-way fp16 split (6 kept cross-product pairs)

_cache: dict = {}
LAST_RUN = None  # BassKernelResults of the most recent launch (for test.py)


def _register_cube_op():
    """Register a fused clamp+cube+scale custom DVE op (the documented
    dve_ops extension point, applied at runtime since the repo is read-only):
    out = min(in0, s0)^3 * s1."""
    import concourse.dve_ops as dve_ops
    from concourse.dve_spec import Spec, Src0, C0, C1, lower, minn, sq
    from concourse.dve_uop import DveOpSpec

    name = "CUBE_CLAMP_SCALE_NB"
    for op in dve_ops.OPS:
        if op.name == name:
            return op

    m = minn(Src0, C0)
    spec = Spec(
        body=sq(m) * m * C1,
        reference=lambda in0, in1, s0, s1, imm2: (
            np.minimum(in0.astype(np.float32), s0) ** 3 * s1
        ),
    )
    row = dve_ops._CUSTOM_DVE_ROW_BASE + len(dve_ops.OPS)
    shas = {}
    for ver in ("v3", "v4"):
        try:
            uops = lower(spec, ver=ver)
        except Exception:
            continue
        shas[ver] = DveOpSpec(name=name, opcode=row, uops=uops, rd1_en=False).sha(ver)
    op = dve_ops.DveOp(name, spec, subdim=False, uops_sha=shas)
    dve_ops.OPS.append(op)
    dve_ops.CUSTOM_DVE_SPECS[name] = spec
    dve_ops._SUB_OPCODE_FOR_NAME[name] = row
    return op


def _build():
    import concourse.bacc as bacc
    import concourse.mybir as mybir
    import concourse.tile as tile

    f32 = mybir.dt.float32
    f16 = mybir.dt.float16
    AF = mybir.ActivationFunctionType

    cube_op = _register_cube_op()

    nc = bacc.Bacc("TRN2", target_bir_lowering=False, debug=False)
    aT = nc.dram_tensor("aT", [KF, N], f16, kind="ExternalInput")
    bT = nc.dram_tensor("bT", [KF, JL], f16, kind="ExternalInput")
    ypc = nc.dram_tensor("ypc", [N, 8], f16, kind="ExternalInput")
    S = nc.dram_tensor("S", [8, JL], f32, kind="ExternalOutput")

    with tile.TileContext(nc) as tc:
        with (
            tc.tile_pool(name="const", bufs=1) as cpool,
            tc.tile_pool(name="tp", bufs=3) as tpool,
            tc.tile_pool(name="icdp", bufs=3) as icdpool,
            tc.tile_pool(name="ps1", bufs=3, space="PSUM") as ps1pool,
            tc.tile_pool(name="ps2", bufs=1, space="PSUM") as ps2pool,
            tc.tile_pool(name="outp", bufs=1) as opool,
        ):
            aT_sb = cpool.tile([KF, N], f16)
            nc.sync.dma_start(aT_sb[:], aT[:])
            bT_sb = cpool.tile([KF, JL], f16)
            nc.sync.dma_start(bT_sb[:], bT[:])
            ypc_sb = cpool.tile([P, ITILES, 8], f16)
            nc.sync.dma_start(ypc_sb[:], ypc.rearrange("(t p) c -> p t c", p=P))

            ps2a = ps2pool.tile([8, 512], f32)
            ps2b = ps2pool.tile([8, 512], f32)

            def emit_mm1(it):
                # ps1 tiles share one pool tag -> the scheduler rotates slots
                ps1 = ps1pool.tile([P, JL], f32, tag="ps1t")
                lhs1 = aT_sb[:, it * P : (it + 1) * P]
                nc.tensor.matmul(
                    ps1[:, 0:512], lhs1, bT_sb[:, 0:512], start=True, stop=True
                )
                nc.tensor.matmul(
                    ps1[:, 512:1024], lhs1, bT_sb[:, 512:1024], start=True, stop=True
                )
                return ps1

            # software pipeline: mm1 runs one iteration ahead so the PE has
            # independent work queued while ACT/DVE produce icd for mm2
            ps1_cur = emit_mm1(0)
            for it in range(ITILES):
                t = tpool.tile([P, JL], f32)
                nc.scalar.activation(t[:], ps1_cur[:], AF.Abs_reciprocal_sqrt)
                icd = icdpool.tile([P, JL], f16)
                nc.vector._custom_dve(
                    cube_op, out=icd[:], in0=t[:], s0=T_CLAMP, s1=1.0 / ICD_SCALE
                )
                if it + 1 < ITILES:
                    ps1_cur = emit_mm1(it + 1)
                first, last = it == 0, it == ITILES - 1
                nc.tensor.matmul(
                    ps2a[:], ypc_sb[:, it, :], icd[:, 0:512], start=first, stop=last
                )
                nc.tensor.matmul(
                    ps2b[:], ypc_sb[:, it, :], icd[:, 512:1024], start=first, stop=last
                )
            S_sb = opool.tile([8, JL], f32)
            nc.vector.tensor_copy(S_sb[:, 0:512], ps2a[:])
            nc.vector.tensor_copy(S_sb[:, 512:1024], ps2b[:])
            nc.sync.dma_start(S[:], S_sb[:])
    nc.compile()
    return nc


def _split16(x):
    hi = x.astype(np.float16)
    lo = (x - hi.astype(np.float32)).astype(np.float16)
    return hi, lo


def _split16_3(x):
    h = x.astype(np.float16)
    r = x - h.astype(np.float32)
    m = r.astype(np.float16)
    l = (r - m.astype(np.float32)).astype(np.float16)
    return h, m, l


def kernel(t, y, masses, G):
    global LAST_RUN
    from concourse.bass_utils import run_bass_kernel_spmd

    y = np.asarray(y, np.float32).reshape(N, 3)
    m = np.asarray(masses, np.float32).reshape(N)
    g = np.float32(np.asarray(G).reshape(()))

    d2 = (y * y).sum(1, dtype=np.float32)
    ones = np.ones(N, np.float32)
    a = np.stack([y[:, 0], y[:, 1], y[:, 2], d2, ones])  # [5, N] fp32
    b = np.stack([-2 * y[:, 0], -2 * y[:, 1], -2 * y[:, 2], ones, d2 + EPS])
    ah, am, al = _split16_3(a)
    bh, bm, bl = _split16_3(b)
    # (ah+am+al).(bh+bm+bl) expanded, keeping pairs whose product can reach
    # ~2^-22 of z: (h,h) (h,m) (m,h) (h,l) (l,h) (m,m); dropped terms < 2^-33.
    aT30 = np.ascontiguousarray(np.concatenate([ah, ah, am, ah, al, am], axis=0))
    bT30_full = np.concatenate([bh, bm, bh, bl, bh, bm], axis=0)  # [30, N]
    yp = np.concatenate([y, ones[:, None]], axis=1)  # [N, 4] fp32
    yph, ypl = _split16(yp)
    ypc = np.ascontiguousarray(np.concatenate([yph, ypl], axis=1))  # [N, 8]

    if "nc" not in _cache:
        _cache["nc"] = _build()
    nc = _cache["nc"]

    in_maps = [
        {
            "aT": aT30,
            "bT": np.ascontiguousarray(bT30_full[:, c * JL : (c + 1) * JL]),
            "ypc": ypc,
        }
        for c in range(NCORES)
    ]
    LAST_RUN = run_bass_kernel_spmd(nc, in_maps, core_ids=list(range(NCORES)))
    S8 = np.concatenate([r["S"] for r in LAST_RUN.results], axis=1)  # [8, N]
    S = S8[0:4] + S8[4:8]
    acc = (np.float32(ICD_SCALE) * g * m)[:, None] * (S[0:3].T - y * S[3][:, None])
    return acc.astype(np.float32)


# revision 15
# speedup vs baseline: 2.8014x; 1.0092x over previous
"""Direct N-body gravitational acceleration on 8 Trainium2 NeuronCores.

Sharding: target-particle axis j split across the 8 cores (1024 targets
per core); every core holds the full (N,3) source positions.

Math (reference):
    z[i,j]   = |y_i - y_j|^2 + eps
    icd[i,j] = z^{-3/2}
    acc[j]   = G*m_j * (sum_i icd[i,j]*y_i  -  y_j * sum_i icd[i,j])

Per-core device pipeline (everything O(N^2) stays on-chip):
    mm1 (PE): z[i,j] = a_i . b_j with a_i=(y_i, d2_i, 1),
        b_j=(-2*y_j, 1, d2_j+eps).  To avoid the 4x-cost fp32 matmul
        path each fp32 feature is 3-way fp16 split and the product
        expanded into 6 cross terms -> one K=30 fp16 matmul with
        fp32-grade accuracy.
    ACT: t = Abs_reciprocal_sqrt(z) = z^{-1/2}  (HW-measured 4.4e-5
        max rel err), one pass.
    DVE (custom fused op): icd/16 = min(t, 100)^3 / 16 written as fp16.
        The clamp bounds fp16 at 62500 (z rounding can dip below eps on
        the diagonal; true off-diagonal pairs sit well above the clamp).
    mm2 (PE, fp16, K=128 per i-tile, PSUM-accumulated over 64 i-tiles):
        S[c,j] += sum_i yp[i,c]*icd[i,j], with yp = [y | 1] split into
        fp16 hi+lo halves packed as one [128, 8] weight (the y_i - y_j
        cancellation must survive quantization) -> one matmul per
        (i-tile, j-half).
Host does the O(N) prep (feature splits) and the O(N) affine combine
    acc[j] = 16*G*m_j*(S[0:3,j] - y_j*S[3,j]) with S = S_hi + S_lo.
"""

import numpy as np

N = 8192
NCORES = 8
JL = N // NCORES  # 1024 local targets per core
P = 128
ITILES = N // P  # 64
EPS = np.float32(0.01 * 0.01)
ICD_SCALE = 16.0  # icd stored as icd/16 in fp16
T_CLAMP = 100.0  # = EPS**-0.5; min(t, clamp)^3/16 = 62500 < fp16 max
KF = 30  # feature rows after 3-way fp16 split (6 kept cross-product pairs)

_cache: dict = {}
LAST_RUN = None  # BassKernelResults of the most recent launch (for test.py)


def _register_cube_op():
    """Register a fused clamp+cube+scale custom DVE op (the documented
    dve_ops extension point, applied at runtime since the repo is read-only):
    out = min(in0, s0)^3 * s1."""
    import concourse.dve_ops as dve_ops
    from concourse.dve_spec import Spec, Src0, C0, C1, lower, minn, sq
    from concourse.dve_uop import DveOpSpec

    name = "CUBE_CLAMP_SCALE_NB"
    for op in dve_ops.OPS:
        if op.name == name:
            return op

    m = minn(Src0, C0)
    spec = Spec(
        body=sq(m) * m * C1,
        reference=lambda in0, in1, s0, s1, imm2: (
            np.minimum(in0.astype(np.float32), s0) ** 3 * s1
        ),
    )
    row = dve_ops._CUSTOM_DVE_ROW_BASE + len(dve_ops.OPS)
    shas = {}
    for ver in ("v3", "v4"):
        try:
            uops = lower(spec, ver=ver)
        except Exception:
            continue
        shas[ver] = DveOpSpec(name=name, opcode=row, uops=uops, rd1_en=False).sha(ver)
    op = dve_ops.DveOp(name, spec, subdim=False, uops_sha=shas)
    dve_ops.OPS.append(op)
    dve_ops.CUSTOM_DVE_SPECS[name] = spec
    dve_ops._SUB_OPCODE_FOR_NAME[name] = row
    return op


def _build():
    import concourse.bacc as bacc
    import concourse.mybir as mybir
    import concourse.tile as tile

    f32 = mybir.dt.float32
    f16 = mybir.dt.float16
    AF = mybir.ActivationFunctionType

    cube_op = _register_cube_op()

    nc = bacc.Bacc("TRN2", target_bir_lowering=False, debug=False)
    aT = nc.dram_tensor("aT", [KF, N], f16, kind="ExternalInput")
    bT = nc.dram_tensor("bT", [KF, JL], f16, kind="ExternalInput")
    ypc = nc.dram_tensor("ypc", [N, 8], f16, kind="ExternalInput")
    S = nc.dram_tensor("S", [8, JL], f32, kind="ExternalOutput")

    with tile.TileContext(nc) as tc:
        with (
            tc.tile_pool(name="const", bufs=1) as cpool,
            tc.tile_pool(name="tp", bufs=4) as tpool,
            tc.tile_pool(name="icdp", bufs=4) as icdpool,
            tc.tile_pool(name="ps1", bufs=3, space="PSUM") as ps1pool,
            tc.tile_pool(name="ps2", bufs=1, space="PSUM") as ps2pool,
            tc.tile_pool(name="outp", bufs=1) as opool,
        ):
            aT_sb = cpool.tile([KF, N], f16)
            nc.sync.dma_start(aT_sb[:], aT[:])
            bT_sb = cpool.tile([KF, JL], f16)
            nc.sync.dma_start(bT_sb[:], bT[:])
            ypc_sb = cpool.tile([P, ITILES, 8], f16)
            nc.sync.dma_start(ypc_sb[:], ypc.rearrange("(t p) c -> p t c", p=P))

            ps2a = ps2pool.tile([8, 512], f32)
            ps2b = ps2pool.tile([8, 512], f32)

            # PE warm-up: ~10us of dense dependency-free matmuls on a zeroed
            # tile trips the HAM clock gate to 8/8 (2.4 GHz) before the main
            # loop; otherwise the whole kernel can run at the cold 1.2 GHz.
            # Output goes to the ps2 banks, which the first real accumulating
            # matmul resets via start=True.
            warm_in = cpool.tile([P, 512], f16)
            nc.vector.memset(warm_in[:], 0.0)
            for w in range(16):
                nc.tensor.matmul(
                    ps2a[:] if w % 2 == 0 else ps2b[:],
                    warm_in[:, 0:8],
                    warm_in[:],
                    start=True,
                    stop=True,
                )

            def emit_mm1(it):
                # ps1 tiles share one pool tag -> the scheduler rotates slots
                ps1 = ps1pool.tile([P, JL], f32, tag="ps1t")
                lhs1 = aT_sb[:, it * P : (it + 1) * P]
                nc.tensor.matmul(
                    ps1[:, 0:512], lhs1, bT_sb[:, 0:512], start=True, stop=True
                )
                nc.tensor.matmul(
                    ps1[:, 512:1024], lhs1, bT_sb[:, 512:1024], start=True, stop=True
                )
                return ps1

            # software pipeline: mm1 runs one iteration ahead so the PE has
            # independent work queued while ACT/DVE produce icd for mm2
            ps1_cur = emit_mm1(0)
            for it in range(ITILES):
                t = tpool.tile([P, JL], f32)
                nc.scalar.activation(t[:], ps1_cur[:], AF.Abs_reciprocal_sqrt)
                icd = icdpool.tile([P, JL], f16)
                nc.vector._custom_dve(
                    cube_op, out=icd[:], in0=t[:], s0=T_CLAMP, s1=1.0 / ICD_SCALE
                )
                if it + 1 < ITILES:
                    ps1_cur = emit_mm1(it + 1)
                first, last = it == 0, it == ITILES - 1
                nc.tensor.matmul(
                    ps2a[:], ypc_sb[:, it, :], icd[:, 0:512], start=first, stop=last
                )
                nc.tensor.matmul(
                    ps2b[:], ypc_sb[:, it, :], icd[:, 512:1024], start=first, stop=last
                )
            S_sb = opool.tile([8, JL], f32)
            nc.vector.tensor_copy(S_sb[:, 0:512], ps2a[:])
            nc.vector.tensor_copy(S_sb[:, 512:1024], ps2b[:])
            nc.sync.dma_start(S[:], S_sb[:])
    nc.compile()
    return nc


def _split16(x):
    hi = x.astype(np.float16)
    lo = (x - hi.astype(np.float32)).astype(np.float16)
    return hi, lo


def _split16_3(x):
    h = x.astype(np.float16)
    r = x - h.astype(np.float32)
    m = r.astype(np.float16)
    l = (r - m.astype(np.float32)).astype(np.float16)
    return h, m, l


def kernel(t, y, masses, G):
    global LAST_RUN
    from concourse.bass_utils import run_bass_kernel_spmd

    y = np.asarray(y, np.float32).reshape(N, 3)
    m = np.asarray(masses, np.float32).reshape(N)
    g = np.float32(np.asarray(G).reshape(()))

    d2 = (y * y).sum(1, dtype=np.float32)
    ones = np.ones(N, np.float32)
    a = np.stack([y[:, 0], y[:, 1], y[:, 2], d2, ones])  # [5, N] fp32
    b = np.stack([-2 * y[:, 0], -2 * y[:, 1], -2 * y[:, 2], ones, d2 + EPS])
    ah, am, al = _split16_3(a)
    bh, bm, bl = _split16_3(b)
    # (ah+am+al).(bh+bm+bl) expanded, keeping pairs whose product can reach
    # ~2^-22 of z: (h,h) (h,m) (m,h) (h,l) (l,h) (m,m); dropped terms < 2^-33.
    aT30 = np.ascontiguousarray(np.concatenate([ah, ah, am, ah, al, am], axis=0))
    bT30_full = np.concatenate([bh, bm, bh, bl, bh, bm], axis=0)  # [30, N]
    yp = np.concatenate([y, ones[:, None]], axis=1)  # [N, 4] fp32
    yph, ypl = _split16(yp)
    ypc = np.ascontiguousarray(np.concatenate([yph, ypl], axis=1))  # [N, 8]

    if "nc" not in _cache:
        _cache["nc"] = _build()
    nc = _cache["nc"]

    in_maps = [
        {
            "aT": aT30,
            "bT": np.ascontiguousarray(bT30_full[:, c * JL : (c + 1) * JL]),
            "ypc": ypc,
        }
        for c in range(NCORES)
    ]
    LAST_RUN = run_bass_kernel_spmd(nc, in_maps, core_ids=list(range(NCORES)))
    S8 = np.concatenate([r["S"] for r in LAST_RUN.results], axis=1)  # [8, N]
    S = S8[0:4] + S8[4:8]
    acc = (np.float32(ICD_SCALE) * g * m)[:, None] * (S[0:3].T - y * S[3][:, None])
    return acc.astype(np.float32)
